# revision 11
# baseline (speedup 1.0000x reference)
"""Trainium2 Bass kernel for nn_Model2_8340826488964 (dense_mlp, recurrent+syncBN).

Model per timestep t (T=512, B=2048, NH=1024, NV=31):
    h = relu((h_prev + emb[x_t]) @ W_hh.T + b_hh)
    BN over batch (training stats), out_t = BN(h) @ W_ho.T + b_ho

Strategy: data-parallel over batch (256 rows/core on 8 cores).
 - Layout: features on partitions (8 f-tiles x 128), batch on free axis (256).
 - Recurrence in bf16.
 - emb-add folded into the matmul via e2 = emb @ W_hh.T and a host-precomputed
   one-hot streamed from HBM: z = h@W_hh.T + onehot.T@e2 + b  (PE-only).
 - BN stats: ACT relu pass emits per-feature sums (accum_out); DVE
   tensor_tensor square + tensor_reduce emits sum-of-squares. Stats for G
   steps are batched into ONE AllReduce, fully overlapped: consumption is
   delayed DELAY steps (DELAY >> G+1) so the collective is never on the
   critical path and never head-of-line-blocks the DVE queue.
 - Output path for step t executes at iteration t+DELAY. BN affine is folded
   into the small output matmul: out_t = (W_ho ⊙ a_t) @ h_t + (W_ho @ c_t +
   b_ho) with a = gamma*rsqrt(var+eps), c = beta - mu*a. The scaled weights
   wsc (128x31 per f-tile, computed on DVE) make mm2 8 matmuls of 256 moving
   cols with 31-col LDWEIGHTS (vs 16 LDW-bound matmuls in the hn
   formulation). The c-term is one tiny per-group matmul d = W_ho @ C
   [31 x G], added as a per-partition scalar in the acc bias-add.
 - Output orientation [NV, T, BC] in DRAM; host transposes to [BC, T, NV].
"""
import sys, os
sys.path.insert(0, "/opt/trn_rl_repo")
import numpy as np
import ml_dtypes

from concourse import bass, bacc, tile, bass_utils
from concourse import mybir
from concourse.bass_interp import get_hw_module

BF16 = ml_dtypes.bfloat16

N_CORES = 8
B, T_FULL, NH, NV = 2048, 512, 1024, 31
BC = B // N_CORES            # 256 batch rows per core
NF = NH // 128               # 8 feature tiles
BN_EPS = 1e-5

G = 8                        # steps per stats-allreduce group
DELAY = 17                   # stats consume delay (steps); >> G+1 so the
                             # collective never stalls any engine queue
D = 20                       # h ring depth (> DELAY + 2)

F32 = mybir.dt.float32
BF = mybir.dt.bfloat16
AF = mybir.ActivationFunctionType
OP = mybir.AluOpType
AX = mybir.AxisListType


def build(T: int, g: int = G, delay: int = DELAY, d: int = D, no_cc: bool = False,
          dbg_h: int = -1):
    assert T % g == 0
    assert d > delay + 2
    nc = bacc.Bacc("TRN2", target_bir_lowering=False, debug=False,
                   enable_asserts=False, num_devices=N_CORES)
    hdump_d = None
    if dbg_h >= 0:
        hdump_d = nc.dram_tensor("hdump", [128, NF * 256], BF,
                                 kind="ExternalOutput").ap()

    whh_d = nc.dram_tensor("whh", [128, 64 * 128], BF, kind="ExternalInput").ap()
    e2_d = nc.dram_tensor("e2t", [NV, NH], BF, kind="ExternalInput").ap()
    whot_d = nc.dram_tensor("whot", [128, NF * NV], BF, kind="ExternalInput").ap()
    bhh_d = nc.dram_tensor("bhh", [128, NF], F32, kind="ExternalInput").ap()
    gamr_d = nc.dram_tensor("gamrep", [128, NF * g], F32, kind="ExternalInput").ap()
    betr_d = nc.dram_tensor("betrep", [128, NF * g], F32, kind="ExternalInput").ap()
    bho_d = nc.dram_tensor("bho31", [NV, 1], F32, kind="ExternalInput").ap()
    oneh_d = nc.dram_tensor("oneh", [NV, T * BC], BF, kind="ExternalInput").ap()
    out_d = nc.dram_tensor("out_shard", [NV, T, BC], F32, kind="ExternalOutput").ap()

    n_groups = T // g
    inv_n = 1.0 / float(B)

    with tile.TileContext(nc) as tc:
        with tc.tile_pool(name="const", bufs=1) as cpool, \
             tc.tile_pool(name="hring", bufs=d) as hpool, \
             tc.tile_pool(name="oneh", bufs=3) as opool, \
             tc.tile_pool(name="stats", bufs=2) as spool, \
             tc.tile_pool(name="fin", bufs=2) as fpool, \
             tc.tile_pool(name="wsc", bufs=2) as wpool, \
             tc.tile_pool(name="acc", bufs=2) as accpool, \
             tc.tile_pool(name="scratch", bufs=1) as scpool, \
             tc.tile_pool(name="ps1", bufs=3, space="PSUM") as ps1pool, \
             tc.tile_pool(name="ps2", bufs=1, space="PSUM") as ps2pool, \
             tc.tile_pool(name="dps", bufs=1, space="PSUM") as dpspool, \
             tc.tile_pool(name="dram", bufs=4, space="DRAM") as dpool:

            # ---- load constants ----
            whh = cpool.tile([128, 64 * 128], BF, tag="whh", name="whh")
            e2 = cpool.tile([NV, NH], BF, tag="e2", name="e2")
            whot = cpool.tile([128, NF * NV], BF, tag="whot", name="whot")
            bhh = cpool.tile([128, NF], F32, tag="bhh", name="bhh")
            gamr = cpool.tile([128, NF * g], F32, tag="gamr", name="gamr")
            betr = cpool.tile([128, NF * g], F32, tag="betr", name="betr")
            bho = cpool.tile([NV, 1], F32, tag="bho", name="bho")
            nc.sync.dma_start(whh[:], whh_d[:])
            nc.sync.dma_start(e2[:], e2_d[:])
            nc.sync.dma_start(whot[:], whot_d[:])
            nc.sync.dma_start(bhh[:], bhh_d[:])
            nc.sync.dma_start(gamr[:], gamr_d[:])
            nc.sync.dma_start(betr[:], betr_d[:])
            nc.sync.dma_start(bho[:], bho_d[:])

            sq_scr = scpool.tile([128, 256], BF, tag="sqscr", name="sqscr")

            h_tiles = {}        # s -> h tile [128, NF*256] bf16
            oneh_tiles = {}     # group -> [NV, g*BC] bf16
            stats_loc = {}      # group -> [128, 16*g] f32 (sums | sumsqs)
            stats_glb = {}      # group -> [128, 16*g] f32
            ac_g = {}           # group -> (A [128,8g] f32, e [NV,g] f32)
            acc_tiles = {}      # group -> [NV, g*BC] f32

            # prefetch onehot for groups 0,1
            for gg in range(min(2, n_groups)):
                ot = opool.tile([NV, g * BC], BF, tag="oneh", name="oneh")
                nc.sync.dma_start(ot[:], oneh_d[:, gg * g * BC:(gg + 1) * g * BC])
                oneh_tiles[gg] = ot

            for s in range(T + delay):
                # ==== early aux work for the delayed path (inputs are all
                # DELAY steps old, so these never wait; emitting them first
                # keeps the in-order DVE/ACT queues from stalling PE's mm2) ==
                t = s - delay
                wsc = None
                if 0 <= t < T:
                    ut, gt = t % g, t // g
                    if ut == 0:
                        # finalize group stats: a = gamma*rsqrt(var+eps),
                        # c = beta - mu*a  (all [128, 8g], u-major cols)
                        sg = stats_glb[gt]
                        mean = fpool.tile([128, 8 * g], F32, tag="mean", name="mean")
                        ex2 = fpool.tile([128, 8 * g], F32, tag="ex2", name="ex2")
                        vep = fpool.tile([128, 8 * g], F32, tag="vep", name="vep")
                        rcp = fpool.tile([128, 8 * g], F32, tag="rcp", name="rcp")
                        rsq = fpool.tile([128, 8 * g], F32, tag="rsq", name="rsq")
                        m2 = fpool.tile([128, 8 * g], F32, tag="m2", name="m2")
                        A = fpool.tile([128, 8 * g], F32, tag="A", name="A")
                        Ct = fpool.tile([128, 8 * g], F32, tag="Ct", name="Ct")
                        C2 = fpool.tile([128, 8 * g], BF, tag="C2", name="C2")
                        nc.vector.tensor_scalar(mean[:], sg[:, 0:8 * g], inv_n, None, OP.mult)
                        nc.vector.tensor_scalar(ex2[:], sg[:, 8 * g:16 * g], inv_n, None, OP.mult)
                        nc.vector.tensor_tensor(m2[:], mean[:], mean[:], OP.mult)
                        nc.vector.scalar_tensor_tensor(
                            vep[:], ex2[:], BN_EPS, m2[:], op0=OP.add, op1=OP.subtract)
                        nc.vector.reciprocal(rcp[:], vep[:])
                        nc.scalar.activation(rsq[:], rcp[:], AF.Sqrt)
                        nc.vector.tensor_tensor(A[:], rsq[:], gamr[:], OP.mult)
                        nc.vector.tensor_tensor(Ct[:], mean[:], A[:], OP.mult)
                        # C2 = (betr - Ct) cast bf16, written fi-major:
                        # C2[:, fi*g + u] = betr[:, u*8+fi] - Ct[:, u*8+fi]
                        nc.vector.tensor_tensor(
                            C2[:].rearrange("p (f u) -> p u f", f=8, u=g),
                            betr[:], Ct[:], OP.subtract)
                        acc_tiles[gt] = accpool.tile([NV, g * BC], F32, tag="acc", name="acc")
                        ac_g[gt] = (A, C2, None)

                    A = ac_g[gt][0]
                    # per-step scaled output weights: wsc_fi = whot_fi * a_fi
                    wsc = wpool.tile([128, NF * NV], BF, tag="wsc", name="wsc")
                    for fi in range(NF):
                        nc.vector.tensor_scalar(
                            wsc[:, fi * NV:(fi + 1) * NV],
                            whot[:, fi * NV:(fi + 1) * NV],
                            A[:, ut * 8 + fi:ut * 8 + fi + 1], None, OP.mult)

                # ======== forward recurrence step s ========
                if s < T:
                    u, gg = s % g, s // g
                    if u == 0:
                        stats_loc[gg] = spool.tile([128, 16 * g], F32, tag="sloc", name="sloc")
                        if gg + 2 < n_groups:
                            ot = opool.tile([NV, g * BC], BF, tag="oneh", name="oneh")
                            nc.sync.dma_start(
                                ot[:], oneh_d[:, (gg + 2) * g * BC:(gg + 3) * g * BC])
                            oneh_tiles[gg + 2] = ot
                    sloc = stats_loc[gg]
                    h_t = hpool.tile([128, NF * 256], BF, tag="h", name="h")
                    h_tiles[s] = h_t
                    h_prev = h_tiles.get(s - 1)
                    oneh_g = oneh_tiles[gg]

                    for half in range(2):
                        psh = ps1pool.tile([128, 1024], F32, tag="ps1", name="ps1")
                        for q in range(4):
                            fi = half * 4 + q
                            pslice = psh[:, q * 256:(q + 1) * 256]
                            if s > 0:
                                for ki in range(NF):
                                    nc.tensor.matmul(
                                        pslice,
                                        whh[:, (ki * NF + fi) * 128:(ki * NF + fi + 1) * 128],
                                        h_prev[:, ki * 256:(ki + 1) * 256],
                                        start=(ki == 0), stop=False)
                            nc.tensor.matmul(
                                pslice,
                                e2[:, fi * 128:(fi + 1) * 128],
                                oneh_g[:, u * BC:(u + 1) * BC],
                                start=(s == 0), stop=True)
                            # ACT: h = relu(psum + b), accum_out = per-feature sum
                            nc.scalar.activation(
                                h_t[:, fi * 256:(fi + 1) * 256], pslice,
                                AF.Relu, bias=bhh[:, fi:fi + 1], scale=1.0,
                                accum_out=sloc[:, u * 8 + fi:u * 8 + fi + 1])
                            # DVE: sum of squares (stt+accum is cheaper than
                            # TT+tensor_reduce: reduce gets no 2x mode)
                            nc.vector.scalar_tensor_tensor(
                                sq_scr[:], h_t[:, fi * 256:(fi + 1) * 256], 1.0,
                                h_t[:, fi * 256:(fi + 1) * 256],
                                op0=OP.mult, op1=OP.mult,
                                accum_out=sloc[:, 8 * g + u * 8 + fi:8 * g + u * 8 + fi + 1])

                    if u == g - 1:
                        # group complete: allreduce the stats (gpsimd queue)
                        cin = dpool.tile([128, 16 * g], F32, tag="ccin", name="ccin")
                        cout = dpool.tile([128, 16 * g], F32, tag="ccout", name="ccout",
                                          addr_space="Shared")
                        nc.gpsimd.dma_start(cin[:], sloc[:])
                        if no_cc:
                            nc.gpsimd.dma_start(cout[:], cin[:])
                        else:
                            nc.gpsimd.collective_compute(
                                "AllReduce", OP.add, ins=[cin[:]], outs=[cout[:]],
                                replica_groups=[list(range(N_CORES))])
                        sg = spool.tile([128, 16 * g], F32, tag="sglb", name="sglb")
                        nc.gpsimd.dma_start(sg[:], cout[:])
                        stats_glb[gg] = sg

                # ======== delayed BN + output path for step t = s-delay ====
                if 0 <= t < T:
                    if ut == 0:
                        # d-matmul: d[v, u] = sum_f W_ho[v, f] * c[f, u]
                        A, C2, _ = ac_g[gt]
                        dps = dpspool.tile([NV, g], F32, tag="dps", name="dps")
                        for fi in range(NF):
                            nc.tensor.matmul(
                                dps[:],
                                whot[:, fi * NV:(fi + 1) * NV],
                                C2[:, fi * g:(fi + 1) * g],
                                start=(fi == 0), stop=(fi == NF - 1))
                        e_t = fpool.tile([NV, g], F32, tag="e", name="e")
                        nc.vector.tensor_scalar(e_t[:], dps[:], bho[:, 0:1], None, OP.add)
                        ac_g[gt] = (A, C2, e_t)

                    e_t = ac_g[gt][2]
                    h_old = h_tiles.pop(t)
                    if dbg_h == t:
                        nc.sync.dma_start(hdump_d[:], h_old[:])
                    ps2 = ps2pool.tile([NV, BC], F32, tag="ps2", name="ps2")
                    for fi in range(NF):
                        nc.tensor.matmul(
                            ps2[:],
                            wsc[:, fi * NV:(fi + 1) * NV],
                            h_old[:, fi * 256:(fi + 1) * 256],
                            start=(fi == 0), stop=(fi == NF - 1))
                    acc = acc_tiles[gt]
                    # acc[:, step] = ps2 + e_u (per-partition scalar)
                    nc.vector.tensor_scalar(
                        acc[:, ut * BC:(ut + 1) * BC], ps2[:],
                        e_t[:, ut:ut + 1], None, OP.add)

                    if ut == g - 1:
                        nc.sync.dma_start(
                            out_d[:, gt * g:(gt + 1) * g, :],
                            acc[:])
                        del acc_tiles[gt], stats_glb[gt], ac_g[gt], stats_loc[gt]
                        if gt in oneh_tiles:
                            del oneh_tiles[gt]

    nc.compile()
    nc.m = get_hw_module(nc.m)
    return nc


def prep_inputs(x, emb, W_hh, b_hh, W_ho, b_ho, gamma, beta, T, g=G):
    """Host-side packing. Returns in_maps (list of per-core dicts)."""
    x = np.asarray(x)
    emb = np.asarray(emb, np.float32)
    W_hh = np.asarray(W_hh, np.float32)
    b_hh = np.asarray(b_hh, np.float32)
    W_ho = np.asarray(W_ho, np.float32)
    b_ho = np.asarray(b_ho, np.float32)
    gamma = np.asarray(gamma, np.float32)
    beta = np.asarray(beta, np.float32)

    WT = np.ascontiguousarray(W_hh.T)                      # [k, f]
    whh = WT.reshape(NF, 128, NF, 128).transpose(1, 0, 2, 3)  # [k_l, ki, fi, f_l]
    whh = np.ascontiguousarray(whh.reshape(128, 64 * 128)).astype(BF16)
    e2 = (emb @ W_hh.T).astype(BF16)                       # [31, 1024]
    whot = np.ascontiguousarray(
        W_ho.T.reshape(NF, 128, NV).transpose(1, 0, 2).reshape(128, NF * NV)).astype(BF16)
    bhh = np.ascontiguousarray(b_hh.reshape(NF, 128).T)    # [128, 8]
    gam = np.ascontiguousarray(gamma.reshape(NF, 128).T)   # [128, 8]
    bet = np.ascontiguousarray(beta.reshape(NF, 128).T)
    gamrep = np.ascontiguousarray(np.tile(gam, (1, g)))    # [128, 8g] u-major
    betrep = np.ascontiguousarray(np.tile(bet, (1, g)))
    bho31 = np.ascontiguousarray(b_ho.reshape(NV, 1))

    common = dict(whh=whh, e2t=e2, whot=whot, bhh=bhh,
                  gamrep=gamrep, betrep=betrep, bho31=bho31)
    in_maps = []
    t_idx = np.arange(T)[:, None]
    b_idx = np.arange(BC)[None, :]
    for c in range(N_CORES):
        xc = x[c * BC:(c + 1) * BC, :T].T                  # [T, 256]
        oh = np.zeros((NV, T, BC), dtype=BF16)
        oh[xc, t_idx, b_idx] = 1
        m = dict(common)
        m["oneh"] = oh.reshape(NV, T * BC)
        in_maps.append(m)
    return in_maps


_CACHE = {}

def _get_built(T):
    if T not in _CACHE:
        _CACHE[T] = build(T)
    return _CACHE[T]


def run(inputs, T=T_FULL, trace=False):
    nc = _get_built(T)
    in_maps = prep_inputs(inputs["x"], inputs["emb"], inputs["W_hh"], inputs["b_hh"],
                          inputs["W_ho"], inputs["b_ho"], inputs["gamma"],
                          inputs["beta"], T)
    res = bass_utils.run_bass_kernel_spmd(
        nc, in_maps, core_ids=list(range(N_CORES)), trace=trace)
    # out_shard is [NV, T, BC]; full output is [B, T, NV]
    out = np.concatenate(
        [np.ascontiguousarray(np.transpose(res.results[c]["out_shard"], (2, 1, 0)))
         for c in range(N_CORES)], axis=0)
    return out, res


def kernel(**inputs) -> np.ndarray:
    out, _ = run(inputs, T=T_FULL, trace=False)
    return out


# revision 14
# speedup vs baseline: 1.0873x; 1.0873x over previous
"""Trainium2 Bass kernel for nn_Model2_8340826488964 (dense_mlp, recurrent+syncBN).

Model per timestep t (T=512, B=2048, NH=1024, NV=31):
    h = relu((h_prev + emb[x_t]) @ W_hh.T + b_hh)
    BN over batch (training stats), out_t = BN(h) @ W_ho.T + b_ho

Strategy: data-parallel over batch (256 rows/core on 8 cores).
 - Layout: features on partitions (8 f-tiles x 128), batch on free axis (256).
 - Recurrence in bf16.
 - emb-add folded into the matmul via e2 = emb @ W_hh.T and a host-precomputed
   one-hot streamed from HBM: z = h@W_hh.T + onehot.T@e2 + b  (PE-only).
 - BN stats: ACT relu pass emits per-feature sums (accum_out); DVE
   tensor_tensor square + tensor_reduce emits sum-of-squares. Stats for G
   steps are batched into ONE AllReduce, fully overlapped: consumption is
   delayed DELAY steps (DELAY >> G+1) so the collective is never on the
   critical path and never head-of-line-blocks the DVE queue.
 - Output path for step t executes at iteration t+DELAY. BN affine is folded
   into the small output matmul: out_t = (W_ho ⊙ a_t) @ h_t + (W_ho @ c_t +
   b_ho) with a = gamma*rsqrt(var+eps), c = beta - mu*a. The scaled weights
   wsc (128x31 per f-tile, computed on DVE) make mm2 8 matmuls of 256 moving
   cols with 31-col LDWEIGHTS (vs 16 LDW-bound matmuls in the hn
   formulation). The c-term is one tiny per-group matmul d = W_ho @ C
   [31 x G], added as a per-partition scalar in the acc bias-add.
 - Output orientation [NV, T, BC] in DRAM; host transposes to [BC, T, NV].
"""
import sys, os
sys.path.insert(0, "/opt/trn_rl_repo")
import numpy as np
import ml_dtypes

from concourse import bass, bacc, tile, bass_utils
from concourse import mybir
from concourse.bass_interp import get_hw_module

BF16 = ml_dtypes.bfloat16

N_CORES = 8
B, T_FULL, NH, NV = 2048, 512, 1024, 31
BC = B // N_CORES            # 256 batch rows per core
NF = NH // 128               # 8 feature tiles
BN_EPS = 1e-5

G = 8                        # steps per stats-allreduce group
DELAY = 17                   # stats consume delay (steps); >> G+1 so the
                             # collective never stalls any engine queue
D = 20                       # h ring depth (> DELAY + 2)

F32 = mybir.dt.float32
BF = mybir.dt.bfloat16
AF = mybir.ActivationFunctionType
OP = mybir.AluOpType
AX = mybir.AxisListType


def build(T: int, g: int = G, delay: int = DELAY, d: int = D, no_cc: bool = False,
          dbg_h: int = -1):
    assert T % g == 0
    assert d > delay + 2
    nc = bacc.Bacc("TRN2", target_bir_lowering=False, debug=False,
                   enable_asserts=False, num_devices=N_CORES)
    hdump_d = None
    if dbg_h >= 0:
        hdump_d = nc.dram_tensor("hdump", [128, NF * 256], BF,
                                 kind="ExternalOutput").ap()

    whh_d = nc.dram_tensor("whh", [128, 64 * 128], BF, kind="ExternalInput").ap()
    e2_d = nc.dram_tensor("e2t", [128, NH], BF, kind="ExternalInput").ap()
    whot_d = nc.dram_tensor("whot", [128, NF * NV], BF, kind="ExternalInput").ap()
    bhh_d = nc.dram_tensor("bhh", [128, NF], F32, kind="ExternalInput").ap()
    gamr_d = nc.dram_tensor("gamrep", [128, NF * g], F32, kind="ExternalInput").ap()
    betr_d = nc.dram_tensor("betrep", [128, NF * g], F32, kind="ExternalInput").ap()
    bho_d = nc.dram_tensor("bho31", [NV, 1], F32, kind="ExternalInput").ap()
    oneh_d = nc.dram_tensor("oneh", [128, T * BC], BF, kind="ExternalInput").ap()
    out_d = nc.dram_tensor("out_shard", [NV, T, BC], F32, kind="ExternalOutput").ap()

    n_groups = T // g
    inv_n = 1.0 / float(B)

    with tile.TileContext(nc) as tc:
        with tc.tile_pool(name="const", bufs=1) as cpool, \
             tc.tile_pool(name="hring", bufs=d) as hpool, \
             tc.tile_pool(name="oneh", bufs=3) as opool, \
             tc.tile_pool(name="stats", bufs=2) as spool, \
             tc.tile_pool(name="fin", bufs=2) as fpool, \
             tc.tile_pool(name="wsc", bufs=2) as wpool, \
             tc.tile_pool(name="acc", bufs=2) as accpool, \
             tc.tile_pool(name="scratch", bufs=1) as scpool, \
             tc.tile_pool(name="ps1", bufs=3, space="PSUM") as ps1pool, \
             tc.tile_pool(name="ps2", bufs=1, space="PSUM") as ps2pool, \
             tc.tile_pool(name="dps", bufs=1, space="PSUM") as dpspool, \
             tc.tile_pool(name="dram", bufs=4, space="DRAM") as dpool:

            # ---- load constants ----
            whh = cpool.tile([128, 64 * 128], BF, tag="whh", name="whh")
            e2 = cpool.tile([128, NH], BF, tag="e2", name="e2")
            whot = cpool.tile([128, NF * NV], BF, tag="whot", name="whot")
            bhh = cpool.tile([128, NF], F32, tag="bhh", name="bhh")
            gamr = cpool.tile([128, NF * g], F32, tag="gamr", name="gamr")
            betr = cpool.tile([128, NF * g], F32, tag="betr", name="betr")
            bho = cpool.tile([NV, 1], F32, tag="bho", name="bho")
            nc.sync.dma_start(whh[:], whh_d[:])
            nc.sync.dma_start(e2[:], e2_d[:])
            nc.sync.dma_start(whot[:], whot_d[:])
            nc.sync.dma_start(bhh[:], bhh_d[:])
            nc.sync.dma_start(gamr[:], gamr_d[:])
            nc.sync.dma_start(betr[:], betr_d[:])
            nc.sync.dma_start(bho[:], bho_d[:])

            sq_scr = scpool.tile([128, 256], BF, tag="sqscr", name="sqscr")

            h_tiles = {}        # s -> h tile [128, NF*256] bf16
            oneh_tiles = {}     # group -> [NV, g*BC] bf16
            stats_loc = {}      # group -> [128, 16*g] f32 (sums | sumsqs)
            stats_glb = {}      # group -> [128, 16*g] f32
            ac_g = {}           # group -> (A [128,8g] f32, e [NV,g] f32)
            acc_tiles = {}      # group -> [NV, g*BC] f32

            # prefetch onehot for groups 0,1
            for gg in range(min(2, n_groups)):
                ot = opool.tile([128, g * BC], BF, tag="oneh", name="oneh")
                nc.sync.dma_start(ot[:], oneh_d[:, gg * g * BC:(gg + 1) * g * BC])
                oneh_tiles[gg] = ot

            for s in range(T + delay):
                # ==== early aux work for the delayed path (inputs are all
                # DELAY steps old, so these never wait; emitting them first
                # keeps the in-order DVE/ACT queues from stalling PE's mm2) ==
                t = s - delay
                wsc = None
                if 0 <= t < T:
                    ut, gt = t % g, t // g
                    if ut == 0:
                        # finalize group stats: a = gamma*rsqrt(var+eps),
                        # c = beta - mu*a  (all [128, 8g], u-major cols)
                        sg = stats_glb[gt]
                        mean = fpool.tile([128, 8 * g], F32, tag="mean", name="mean")
                        ex2 = fpool.tile([128, 8 * g], F32, tag="ex2", name="ex2")
                        vep = fpool.tile([128, 8 * g], F32, tag="vep", name="vep")
                        rcp = fpool.tile([128, 8 * g], F32, tag="rcp", name="rcp")
                        rsq = fpool.tile([128, 8 * g], F32, tag="rsq", name="rsq")
                        m2 = fpool.tile([128, 8 * g], F32, tag="m2", name="m2")
                        A = fpool.tile([128, 8 * g], F32, tag="A", name="A")
                        Ct = fpool.tile([128, 8 * g], F32, tag="Ct", name="Ct")
                        C2 = fpool.tile([128, 8 * g], BF, tag="C2", name="C2")
                        nc.vector.tensor_scalar(mean[:], sg[:, 0:8 * g], inv_n, None, OP.mult)
                        nc.vector.tensor_scalar(ex2[:], sg[:, 8 * g:16 * g], inv_n, None, OP.mult)
                        nc.vector.tensor_tensor(m2[:], mean[:], mean[:], OP.mult)
                        nc.vector.scalar_tensor_tensor(
                            vep[:], ex2[:], BN_EPS, m2[:], op0=OP.add, op1=OP.subtract)
                        nc.vector.reciprocal(rcp[:], vep[:])
                        nc.scalar.activation(rsq[:], rcp[:], AF.Sqrt)
                        nc.vector.tensor_tensor(A[:], rsq[:], gamr[:], OP.mult)
                        nc.vector.tensor_tensor(Ct[:], mean[:], A[:], OP.mult)
                        # C2 = (betr - Ct) cast bf16, written fi-major:
                        # C2[:, fi*g + u] = betr[:, u*8+fi] - Ct[:, u*8+fi]
                        nc.vector.tensor_tensor(
                            C2[:].rearrange("p (f u) -> p u f", f=8, u=g),
                            betr[:], Ct[:], OP.subtract)
                        acc_tiles[gt] = accpool.tile([NV, g * BC], F32, tag="acc", name="acc")
                        ac_g[gt] = (A, C2, None)

                    A = ac_g[gt][0]
                    # per-step scaled output weights: wsc_fi = whot_fi * a_fi
                    wsc = wpool.tile([128, NF * NV], BF, tag="wsc", name="wsc")
                    for fi in range(NF):
                        nc.vector.tensor_scalar(
                            wsc[:, fi * NV:(fi + 1) * NV],
                            whot[:, fi * NV:(fi + 1) * NV],
                            A[:, ut * 8 + fi:ut * 8 + fi + 1], None, OP.mult)

                # ======== forward recurrence step s ========
                if s < T:
                    u, gg = s % g, s // g
                    if u == 0:
                        stats_loc[gg] = spool.tile([128, 16 * g], F32, tag="sloc", name="sloc")
                        if gg + 2 < n_groups:
                            ot = opool.tile([128, g * BC], BF, tag="oneh", name="oneh")
                            nc.sync.dma_start(
                                ot[:], oneh_d[:, (gg + 2) * g * BC:(gg + 3) * g * BC])
                            oneh_tiles[gg + 2] = ot
                    sloc = stats_loc[gg]
                    h_t = hpool.tile([128, NF * 256], BF, tag="h", name="h")
                    h_tiles[s] = h_t
                    h_prev = h_tiles.get(s - 1)
                    oneh_g = oneh_tiles[gg]

                    for half in range(2):
                        psh = ps1pool.tile([128, 1024], F32, tag="ps1", name="ps1")
                        for q in range(4):
                            fi = half * 4 + q
                            pslice = psh[:, q * 256:(q + 1) * 256]
                            if s > 0:
                                for ki in range(NF):
                                    nc.tensor.matmul(
                                        pslice,
                                        whh[:, (ki * NF + fi) * 128:(ki * NF + fi + 1) * 128],
                                        h_prev[:, ki * 256:(ki + 1) * 256],
                                        start=(ki == 0), stop=False)
                            nc.tensor.matmul(
                                pslice,
                                e2[:, fi * 128:(fi + 1) * 128],
                                oneh_g[:, u * BC:(u + 1) * BC],
                                start=(s == 0), stop=True)
                            # ACT: h = relu(psum + b), accum_out = per-feature sum
                            nc.scalar.activation(
                                h_t[:, fi * 256:(fi + 1) * 256], pslice,
                                AF.Relu, bias=bhh[:, fi:fi + 1], scale=1.0,
                                accum_out=sloc[:, u * 8 + fi:u * 8 + fi + 1])
                            # DVE: sum of squares (stt+accum is cheaper than
                            # TT+tensor_reduce: reduce gets no 2x mode)
                            nc.vector.scalar_tensor_tensor(
                                sq_scr[:], h_t[:, fi * 256:(fi + 1) * 256], 1.0,
                                h_t[:, fi * 256:(fi + 1) * 256],
                                op0=OP.mult, op1=OP.mult,
                                accum_out=sloc[:, 8 * g + u * 8 + fi:8 * g + u * 8 + fi + 1])

                    if u == g - 1:
                        # group complete: allreduce the stats (gpsimd queue)
                        cin = dpool.tile([128, 16 * g], F32, tag="ccin", name="ccin")
                        cout = dpool.tile([128, 16 * g], F32, tag="ccout", name="ccout",
                                          addr_space="Shared")
                        nc.gpsimd.dma_start(cin[:], sloc[:])
                        if no_cc:
                            nc.gpsimd.dma_start(cout[:], cin[:])
                        else:
                            nc.gpsimd.collective_compute(
                                "AllReduce", OP.add, ins=[cin[:]], outs=[cout[:]],
                                replica_groups=[list(range(N_CORES))])
                        sg = spool.tile([128, 16 * g], F32, tag="sglb", name="sglb")
                        nc.gpsimd.dma_start(sg[:], cout[:])
                        stats_glb[gg] = sg

                # ======== delayed BN + output path for step t = s-delay ====
                if 0 <= t < T:
                    if ut == 0:
                        # d-matmul: d[v, u] = sum_f W_ho[v, f] * c[f, u]
                        A, C2, _ = ac_g[gt]
                        dps = dpspool.tile([NV, g], F32, tag="dps", name="dps")
                        for fi in range(NF):
                            nc.tensor.matmul(
                                dps[:],
                                whot[:, fi * NV:(fi + 1) * NV],
                                C2[:, fi * g:(fi + 1) * g],
                                start=(fi == 0), stop=(fi == NF - 1))
                        e_t = fpool.tile([NV, g], F32, tag="e", name="e")
                        nc.vector.tensor_scalar(e_t[:], dps[:], bho[:, 0:1], None, OP.add)
                        ac_g[gt] = (A, C2, e_t)

                    e_t = ac_g[gt][2]
                    h_old = h_tiles.pop(t)
                    if dbg_h == t:
                        nc.sync.dma_start(hdump_d[:], h_old[:])
                    ps2 = ps2pool.tile([NV, BC], F32, tag="ps2", name="ps2")
                    for fi in range(NF):
                        nc.tensor.matmul(
                            ps2[:],
                            wsc[:, fi * NV:(fi + 1) * NV],
                            h_old[:, fi * 256:(fi + 1) * 256],
                            start=(fi == 0), stop=(fi == NF - 1))
                    acc = acc_tiles[gt]
                    # acc[:, step] = ps2 + e_u (per-partition scalar)
                    nc.vector.tensor_scalar(
                        acc[:, ut * BC:(ut + 1) * BC], ps2[:],
                        e_t[:, ut:ut + 1], None, OP.add)

                    if ut == g - 1:
                        nc.sync.dma_start(
                            out_d[:, gt * g:(gt + 1) * g, :],
                            acc[:])
                        del acc_tiles[gt], stats_glb[gt], ac_g[gt], stats_loc[gt]
                        if gt in oneh_tiles:
                            del oneh_tiles[gt]

    nc.compile()
    nc.m = get_hw_module(nc.m)
    return nc


def prep_inputs(x, emb, W_hh, b_hh, W_ho, b_ho, gamma, beta, T, g=G):
    """Host-side packing. Returns in_maps (list of per-core dicts)."""
    x = np.asarray(x)
    emb = np.asarray(emb, np.float32)
    W_hh = np.asarray(W_hh, np.float32)
    b_hh = np.asarray(b_hh, np.float32)
    W_ho = np.asarray(W_ho, np.float32)
    b_ho = np.asarray(b_ho, np.float32)
    gamma = np.asarray(gamma, np.float32)
    beta = np.asarray(beta, np.float32)

    WT = np.ascontiguousarray(W_hh.T)                      # [k, f]
    whh = WT.reshape(NF, 128, NF, 128).transpose(1, 0, 2, 3)  # [k_l, ki, fi, f_l]
    whh = np.ascontiguousarray(whh.reshape(128, 64 * 128)).astype(BF16)
    # e2 padded to 128 rows (rows 31..127 zero) so the onehot injection is a
    # full-K matmul: partial-row-group LDWEIGHTS can't use the background
    # weight buffer and serializes (~+100ns per matmul).
    e2 = np.zeros((128, NH), dtype=BF16)
    e2[:NV] = (emb @ W_hh.T).astype(BF16)
    whot = np.ascontiguousarray(
        W_ho.T.reshape(NF, 128, NV).transpose(1, 0, 2).reshape(128, NF * NV)).astype(BF16)
    bhh = np.ascontiguousarray(b_hh.reshape(NF, 128).T)    # [128, 8]
    gam = np.ascontiguousarray(gamma.reshape(NF, 128).T)   # [128, 8]
    bet = np.ascontiguousarray(beta.reshape(NF, 128).T)
    gamrep = np.ascontiguousarray(np.tile(gam, (1, g)))    # [128, 8g] u-major
    betrep = np.ascontiguousarray(np.tile(bet, (1, g)))
    bho31 = np.ascontiguousarray(b_ho.reshape(NV, 1))

    common = dict(whh=whh, e2t=e2, whot=whot, bhh=bhh,
                  gamrep=gamrep, betrep=betrep, bho31=bho31)
    in_maps = []
    t_idx = np.arange(T)[:, None]
    b_idx = np.arange(BC)[None, :]
    for c in range(N_CORES):
        xc = x[c * BC:(c + 1) * BC, :T].T                  # [T, 256]
        oh = np.zeros((128, T, BC), dtype=BF16)            # padded rows 31..127
        oh[xc, t_idx, b_idx] = 1
        m = dict(common)
        m["oneh"] = oh.reshape(128, T * BC)
        in_maps.append(m)
    return in_maps


_CACHE = {}

def _get_built(T):
    if T not in _CACHE:
        _CACHE[T] = build(T)
    return _CACHE[T]


def run(inputs, T=T_FULL, trace=False):
    nc = _get_built(T)
    in_maps = prep_inputs(inputs["x"], inputs["emb"], inputs["W_hh"], inputs["b_hh"],
                          inputs["W_ho"], inputs["b_ho"], inputs["gamma"],
                          inputs["beta"], T)
    res = bass_utils.run_bass_kernel_spmd(
        nc, in_maps, core_ids=list(range(N_CORES)), trace=trace)
    # out_shard is [NV, T, BC]; full output is [B, T, NV]
    out = np.concatenate(
        [np.ascontiguousarray(np.transpose(res.results[c]["out_shard"], (2, 1, 0)))
         for c in range(N_CORES)], axis=0)
    return out, res


def kernel(**inputs) -> np.ndarray:
    out, _ = run(inputs, T=T_FULL, trace=False)
    return out


# revision 15
# speedup vs baseline: 1.1160x; 1.0264x over previous
"""Trainium2 Bass kernel for nn_Model2_8340826488964 (dense_mlp, recurrent+syncBN).

Model per timestep t (T=512, B=2048, NH=1024, NV=31):
    h = relu((h_prev + emb[x_t]) @ W_hh.T + b_hh)
    BN over batch (training stats), out_t = BN(h) @ W_ho.T + b_ho

Strategy: data-parallel over batch (256 rows/core on 8 cores).
 - Layout: features on partitions (8 f-tiles x 128), batch on free axis (256).
 - Recurrence in bf16.
 - emb-add folded into the matmul via e2 = emb @ W_hh.T and a host-precomputed
   one-hot streamed from HBM: z = h@W_hh.T + onehot.T@e2 + b  (PE-only).
 - BN stats: ACT relu pass emits per-feature sums (accum_out); DVE
   tensor_tensor square + tensor_reduce emits sum-of-squares. Stats for G
   steps are batched into ONE AllReduce, fully overlapped: consumption is
   delayed DELAY steps (DELAY >> G+1) so the collective is never on the
   critical path and never head-of-line-blocks the DVE queue.
 - Output path for step t executes at iteration t+DELAY. BN affine is folded
   into the small output matmul: out_t = (W_ho ⊙ a_t) @ h_t + (W_ho @ c_t +
   b_ho) with a = gamma*rsqrt(var+eps), c = beta - mu*a. The scaled weights
   wsc (128x31 per f-tile, computed on DVE) make mm2 8 matmuls of 256 moving
   cols with 31-col LDWEIGHTS (vs 16 LDW-bound matmuls in the hn
   formulation). The c-term is one tiny per-group matmul d = W_ho @ C
   [31 x G], added as a per-partition scalar in the acc bias-add.
 - Output orientation [NV, T, BC] in DRAM; host transposes to [BC, T, NV].
"""
import sys, os
sys.path.insert(0, "/opt/trn_rl_repo")
import numpy as np
import ml_dtypes

from concourse import bass, bacc, tile, bass_utils
from concourse import mybir
from concourse.bass_interp import get_hw_module

BF16 = ml_dtypes.bfloat16

N_CORES = 8
B, T_FULL, NH, NV = 2048, 512, 1024, 31
BC = B // N_CORES            # 256 batch rows per core
NF = NH // 128               # 8 feature tiles
BN_EPS = 1e-5

G = 8                        # steps per stats-allreduce group
DELAY = 17                   # stats consume delay (steps); >> G+1 so the
                             # collective never stalls any engine queue
D = 20                       # h ring depth (> DELAY + 2)

F32 = mybir.dt.float32
BF = mybir.dt.bfloat16
AF = mybir.ActivationFunctionType
OP = mybir.AluOpType
AX = mybir.AxisListType


def build(T: int, g: int = G, delay: int = DELAY, d: int = D, no_cc: bool = False,
          dbg_h: int = -1):
    assert T % g == 0
    assert d > delay + 2
    nc = bacc.Bacc("TRN2", target_bir_lowering=False, debug=False,
                   enable_asserts=False, num_devices=N_CORES)
    hdump_d = None
    if dbg_h >= 0:
        hdump_d = nc.dram_tensor("hdump", [128, NF * 256], BF,
                                 kind="ExternalOutput").ap()

    whh_d = nc.dram_tensor("whh", [128, 64 * 128], BF, kind="ExternalInput").ap()
    e2_d = nc.dram_tensor("e2t", [128, NH], BF, kind="ExternalInput").ap()
    whot_d = nc.dram_tensor("whot", [128, NF * 128], BF, kind="ExternalInput").ap()
    bhh_d = nc.dram_tensor("bhh", [128, NF], F32, kind="ExternalInput").ap()
    gamr_d = nc.dram_tensor("gamrep", [128, NF * g], F32, kind="ExternalInput").ap()
    betr_d = nc.dram_tensor("betrep", [128, NF * g], F32, kind="ExternalInput").ap()
    bho_d = nc.dram_tensor("bho31", [NV, 1], F32, kind="ExternalInput").ap()
    oneh_d = nc.dram_tensor("oneh", [128, T * BC], BF, kind="ExternalInput").ap()
    out_d = nc.dram_tensor("out_shard", [NV, T, BC], F32, kind="ExternalOutput").ap()

    n_groups = T // g
    inv_n = 1.0 / float(B)

    with tile.TileContext(nc) as tc:
        with tc.tile_pool(name="const", bufs=1) as cpool, \
             tc.tile_pool(name="hring", bufs=d) as hpool, \
             tc.tile_pool(name="oneh", bufs=3) as opool, \
             tc.tile_pool(name="stats", bufs=2) as spool, \
             tc.tile_pool(name="fin", bufs=2) as fpool, \
             tc.tile_pool(name="wsc", bufs=3) as wpool, \
             tc.tile_pool(name="acc", bufs=2) as accpool, \
             tc.tile_pool(name="scratch", bufs=1) as scpool, \
             tc.tile_pool(name="ps1", bufs=3, space="PSUM") as ps1pool, \
             tc.tile_pool(name="ps2", bufs=1, space="PSUM") as ps2pool, \
             tc.tile_pool(name="dps", bufs=1, space="PSUM") as dpspool, \
             tc.tile_pool(name="dram", bufs=4, space="DRAM") as dpool:

            # ---- load constants ----
            whh = cpool.tile([128, 64 * 128], BF, tag="whh", name="whh")
            e2 = cpool.tile([128, NH], BF, tag="e2", name="e2")
            whot = cpool.tile([128, NF * 128], BF, tag="whot", name="whot")
            bhh = cpool.tile([128, NF], F32, tag="bhh", name="bhh")
            gamr = cpool.tile([128, NF * g], F32, tag="gamr", name="gamr")
            betr = cpool.tile([128, NF * g], F32, tag="betr", name="betr")
            bho = cpool.tile([NV, 1], F32, tag="bho", name="bho")
            nc.sync.dma_start(whh[:], whh_d[:])
            nc.sync.dma_start(e2[:], e2_d[:])
            nc.sync.dma_start(whot[:], whot_d[:])
            nc.sync.dma_start(bhh[:], bhh_d[:])
            nc.sync.dma_start(gamr[:], gamr_d[:])
            nc.sync.dma_start(betr[:], betr_d[:])
            nc.sync.dma_start(bho[:], bho_d[:])

            sq_scr = scpool.tile([128, 256], BF, tag="sqscr", name="sqscr")

            h_tiles = {}        # s -> h tile [128, NF*256] bf16
            oneh_tiles = {}     # group -> [NV, g*BC] bf16
            stats_loc = {}      # group -> [128, 16*g] f32 (sums | sumsqs)
            stats_glb = {}      # group -> [128, 16*g] f32
            ac_g = {}           # group -> (A [128,8g] f32, e [NV,g] f32)
            acc_tiles = {}      # group -> [NV, g*BC] f32

            # prefetch onehot for groups 0,1
            for gg in range(min(2, n_groups)):
                ot = opool.tile([128, g * BC], BF, tag="oneh", name="oneh")
                nc.sync.dma_start(ot[:], oneh_d[:, gg * g * BC:(gg + 1) * g * BC])
                oneh_tiles[gg] = ot

            for s in range(T + delay):
                # ==== early aux work for the delayed path (inputs are all
                # DELAY steps old, so these never wait; emitting them first
                # keeps the in-order DVE/ACT queues from stalling PE's mm2) ==
                t = s - delay
                wsc = None
                if 0 <= t < T:
                    ut, gt = t % g, t // g
                    if ut == 0:
                        # finalize group stats: a = gamma*rsqrt(var+eps),
                        # c = beta - mu*a  (all [128, 8g], u-major cols)
                        sg = stats_glb[gt]
                        mean = fpool.tile([128, 8 * g], F32, tag="mean", name="mean")
                        ex2 = fpool.tile([128, 8 * g], F32, tag="ex2", name="ex2")
                        vep = fpool.tile([128, 8 * g], F32, tag="vep", name="vep")
                        rcp = fpool.tile([128, 8 * g], F32, tag="rcp", name="rcp")
                        rsq = fpool.tile([128, 8 * g], F32, tag="rsq", name="rsq")
                        m2 = fpool.tile([128, 8 * g], F32, tag="m2", name="m2")
                        A = fpool.tile([128, 8 * g], F32, tag="A", name="A")
                        Ct = fpool.tile([128, 8 * g], F32, tag="Ct", name="Ct")
                        C2 = fpool.tile([128, 8 * g], BF, tag="C2", name="C2")
                        nc.vector.tensor_scalar(mean[:], sg[:, 0:8 * g], inv_n, None, OP.mult)
                        nc.vector.tensor_scalar(ex2[:], sg[:, 8 * g:16 * g], inv_n, None, OP.mult)
                        nc.vector.tensor_tensor(m2[:], mean[:], mean[:], OP.mult)
                        nc.vector.scalar_tensor_tensor(
                            vep[:], ex2[:], BN_EPS, m2[:], op0=OP.add, op1=OP.subtract)
                        nc.vector.reciprocal(rcp[:], vep[:])
                        nc.scalar.activation(rsq[:], rcp[:], AF.Sqrt)
                        nc.vector.tensor_tensor(A[:], rsq[:], gamr[:], OP.mult)
                        nc.vector.tensor_tensor(Ct[:], mean[:], A[:], OP.mult)
                        # C2 = (betr - Ct) cast bf16, written fi-major:
                        # C2[:, fi*g + u] = betr[:, u*8+fi] - Ct[:, u*8+fi]
                        nc.vector.tensor_tensor(
                            C2[:].rearrange("p (f u) -> p u f", f=8, u=g),
                            betr[:], Ct[:], OP.subtract)
                        acc_tiles[gt] = accpool.tile([NV, g * BC], F32, tag="acc", name="acc")
                        ac_g[gt] = (A, C2, None)

                    A = ac_g[gt][0]
                    # per-step scaled output weights: wsc_fi = whot_fi * a_fi
                    wsc = wpool.tile([128, NF * 128], BF, tag="wsc", name="wsc")
                    for fi in range(NF):
                        nc.vector.tensor_scalar(
                            wsc[:, fi * 128:(fi + 1) * 128],
                            whot[:, fi * 128:(fi + 1) * 128],
                            A[:, ut * 8 + fi:ut * 8 + fi + 1], None, OP.mult)

                # ======== forward recurrence step s ========
                if s < T:
                    u, gg = s % g, s // g
                    if u == 0:
                        stats_loc[gg] = spool.tile([128, 16 * g], F32, tag="sloc", name="sloc")
                        if gg + 2 < n_groups:
                            ot = opool.tile([128, g * BC], BF, tag="oneh", name="oneh")
                            nc.sync.dma_start(
                                ot[:], oneh_d[:, (gg + 2) * g * BC:(gg + 3) * g * BC])
                            oneh_tiles[gg + 2] = ot
                    sloc = stats_loc[gg]
                    h_t = hpool.tile([128, NF * 256], BF, tag="h", name="h")
                    h_tiles[s] = h_t
                    h_prev = h_tiles.get(s - 1)
                    oneh_g = oneh_tiles[gg]

                    for half in range(2):
                        psh = ps1pool.tile([128, 1024], F32, tag="ps1", name="ps1")
                        for q in range(4):
                            fi = half * 4 + q
                            pslice = psh[:, q * 256:(q + 1) * 256]
                            if s > 0:
                                for ki in range(NF):
                                    nc.tensor.matmul(
                                        pslice,
                                        whh[:, (ki * NF + fi) * 128:(ki * NF + fi + 1) * 128],
                                        h_prev[:, ki * 256:(ki + 1) * 256],
                                        start=(ki == 0), stop=False)
                            nc.tensor.matmul(
                                pslice,
                                e2[:, fi * 128:(fi + 1) * 128],
                                oneh_g[:, u * BC:(u + 1) * BC],
                                start=(s == 0), stop=True)
                            # ACT: h = relu(psum + b), accum_out = per-feature sum
                            nc.scalar.activation(
                                h_t[:, fi * 256:(fi + 1) * 256], pslice,
                                AF.Relu, bias=bhh[:, fi:fi + 1], scale=1.0,
                                accum_out=sloc[:, u * 8 + fi:u * 8 + fi + 1])
                            # DVE: sum of squares (stt+accum is cheaper than
                            # TT+tensor_reduce: reduce gets no 2x mode)
                            nc.vector.scalar_tensor_tensor(
                                sq_scr[:], h_t[:, fi * 256:(fi + 1) * 256], 1.0,
                                h_t[:, fi * 256:(fi + 1) * 256],
                                op0=OP.mult, op1=OP.mult,
                                accum_out=sloc[:, 8 * g + u * 8 + fi:8 * g + u * 8 + fi + 1])

                    if u == g - 1:
                        # group complete: allreduce the stats (gpsimd queue)
                        cin = dpool.tile([128, 16 * g], F32, tag="ccin", name="ccin")
                        cout = dpool.tile([128, 16 * g], F32, tag="ccout", name="ccout",
                                          addr_space="Shared")
                        nc.gpsimd.dma_start(cin[:], sloc[:])
                        if no_cc:
                            nc.gpsimd.dma_start(cout[:], cin[:])
                        else:
                            nc.gpsimd.collective_compute(
                                "AllReduce", OP.add, ins=[cin[:]], outs=[cout[:]],
                                replica_groups=[list(range(N_CORES))])
                        sg = spool.tile([128, 16 * g], F32, tag="sglb", name="sglb")
                        nc.gpsimd.dma_start(sg[:], cout[:])
                        stats_glb[gg] = sg

                # ======== delayed BN + output path for step t = s-delay ====
                if 0 <= t < T:
                    if ut == 0:
                        # d-matmul: d[v, u] = sum_f W_ho[v, f] * c[f, u]
                        A, C2, _ = ac_g[gt]
                        dps = dpspool.tile([128, g], F32, tag="dps", name="dps")
                        for fi in range(NF):
                            nc.tensor.matmul(
                                dps[:],
                                whot[:, fi * 128:(fi + 1) * 128],
                                C2[:, fi * g:(fi + 1) * g],
                                start=(fi == 0), stop=(fi == NF - 1))
                        e_t = fpool.tile([NV, g], F32, tag="e", name="e")
                        nc.vector.tensor_scalar(e_t[:], dps[0:NV, :], bho[:, 0:1], None, OP.add)
                        ac_g[gt] = (A, C2, e_t)

                    e_t = ac_g[gt][2]
                    h_old = h_tiles.pop(t)
                    if dbg_h == t:
                        nc.sync.dma_start(hdump_d[:], h_old[:])
                    ps2 = ps2pool.tile([128, BC], F32, tag="ps2", name="ps2")
                    for fi in range(NF):
                        nc.tensor.matmul(
                            ps2[:],
                            wsc[:, fi * 128:(fi + 1) * 128],
                            h_old[:, fi * 256:(fi + 1) * 256],
                            start=(fi == 0), stop=(fi == NF - 1))
                    acc = acc_tiles[gt]
                    # acc[:, step] = ps2 + e_u (per-partition scalar)
                    nc.vector.tensor_scalar(
                        acc[:, ut * BC:(ut + 1) * BC], ps2[0:NV, :],
                        e_t[:, ut:ut + 1], None, OP.add)

                    if ut == g - 1:
                        nc.sync.dma_start(
                            out_d[:, gt * g:(gt + 1) * g, :],
                            acc[:])
                        del acc_tiles[gt], stats_glb[gt], ac_g[gt], stats_loc[gt]
                        if gt in oneh_tiles:
                            del oneh_tiles[gt]

    nc.compile()
    nc.m = get_hw_module(nc.m)
    return nc


def prep_inputs(x, emb, W_hh, b_hh, W_ho, b_ho, gamma, beta, T, g=G):
    """Host-side packing. Returns in_maps (list of per-core dicts)."""
    x = np.asarray(x)
    emb = np.asarray(emb, np.float32)
    W_hh = np.asarray(W_hh, np.float32)
    b_hh = np.asarray(b_hh, np.float32)
    W_ho = np.asarray(W_ho, np.float32)
    b_ho = np.asarray(b_ho, np.float32)
    gamma = np.asarray(gamma, np.float32)
    beta = np.asarray(beta, np.float32)

    WT = np.ascontiguousarray(W_hh.T)                      # [k, f]
    whh = WT.reshape(NF, 128, NF, 128).transpose(1, 0, 2, 3)  # [k_l, ki, fi, f_l]
    whh = np.ascontiguousarray(whh.reshape(128, 64 * 128)).astype(BF16)
    # e2 padded to 128 rows (rows 31..127 zero) so the onehot injection is a
    # full-K matmul: partial-row-group LDWEIGHTS can't use the background
    # weight buffer and serializes (~+100ns per matmul).
    e2 = np.zeros((128, NH), dtype=BF16)
    e2[:NV] = (emb @ W_hh.T).astype(BF16)
    # whot padded to M=128 per f-tile (cols 31..127 zero): M<32 matmuls hit a
    # col_grp=q0 <-> full-array transition penalty (~100ns) on every boundary.
    whot = np.zeros((128, NF, 128), dtype=BF16)
    whot[:, :, :NV] = W_ho.T.reshape(NF, 128, NV).transpose(1, 0, 2).astype(BF16)
    whot = np.ascontiguousarray(whot.reshape(128, NF * 128))
    bhh = np.ascontiguousarray(b_hh.reshape(NF, 128).T)    # [128, 8]
    gam = np.ascontiguousarray(gamma.reshape(NF, 128).T)   # [128, 8]
    bet = np.ascontiguousarray(beta.reshape(NF, 128).T)
    gamrep = np.ascontiguousarray(np.tile(gam, (1, g)))    # [128, 8g] u-major
    betrep = np.ascontiguousarray(np.tile(bet, (1, g)))
    bho31 = np.ascontiguousarray(b_ho.reshape(NV, 1))

    common = dict(whh=whh, e2t=e2, whot=whot, bhh=bhh,
                  gamrep=gamrep, betrep=betrep, bho31=bho31)
    in_maps = []
    t_idx = np.arange(T)[:, None]
    b_idx = np.arange(BC)[None, :]
    for c in range(N_CORES):
        xc = x[c * BC:(c + 1) * BC, :T].T                  # [T, 256]
        oh = np.zeros((128, T, BC), dtype=BF16)            # padded rows 31..127
        oh[xc, t_idx, b_idx] = 1
        m = dict(common)
        m["oneh"] = oh.reshape(128, T * BC)
        in_maps.append(m)
    return in_maps


_CACHE = {}

def _get_built(T):
    if T not in _CACHE:
        _CACHE[T] = build(T)
    return _CACHE[T]


def run(inputs, T=T_FULL, trace=False):
    nc = _get_built(T)
    in_maps = prep_inputs(inputs["x"], inputs["emb"], inputs["W_hh"], inputs["b_hh"],
                          inputs["W_ho"], inputs["b_ho"], inputs["gamma"],
                          inputs["beta"], T)
    res = bass_utils.run_bass_kernel_spmd(
        nc, in_maps, core_ids=list(range(N_CORES)), trace=trace)
    # out_shard is [NV, T, BC]; full output is [B, T, NV]
    out = np.concatenate(
        [np.ascontiguousarray(np.transpose(res.results[c]["out_shard"], (2, 1, 0)))
         for c in range(N_CORES)], axis=0)
    return out, res


def kernel(**inputs) -> np.ndarray:
    out, _ = run(inputs, T=T_FULL, trace=False)
    return out
